# revision 20
# baseline (speedup 1.0000x reference)
"""Chamfer loss kernel for 8x TRN2 NeuronCores.

Problem: gts (8, 8192, 3) f32, preds (8, 8192, 3) f32 ->
    scalar = mean_n min_m d2[b,n,m] + mean_m min_n d2[b,n,m]
where d2 = squared euclidean distance.

Sharding: data-parallel over batch B=8, one batch element per core. Each
core evaluates its full 8192x8192 distance block and reduces it to two
partial sums (sum of row-mins, sum of col-mins); the host sums the 8x2
partials and divides by B*N.

Device algorithm (per core):
  - d2 is produced by ONE bf16 matmul per (128n x 512m) tile using an
    augmented split-bf16 embedding with contract dim K=16:
        ahat = [a_hi(3), a_lo(3), a_hi(3), a_lo(3), na_hi, na_lo, 1, 1]
        bhat = [q_hi(3), q_hi(3), q_lo(3), q_lo(3), 1, 1, nb_hi, nb_lo]
    with q = -2*preds, na = |a|^2, nb = |b|^2, each value split into
    hi/lo bf16 parts.  ahat.T @ bhat = |a|^2 + |b|^2 - 2 a.b to ~2^-16
    relative accuracy (PSUM accumulates in f32), i.e. f32-grade d2 at
    bf16 streaming speed.
  - ScalarE copies each PSUM group (128 x 2048 f32) to SBUF as bf16.
  - VectorE does both min reductions in bf16 (tensor_tensor min = 2x
    mode): a running col-min accumulator RM (128 x 8192) folded over
    n-tiles, and a per-n-tile row accumulator R folded over m-groups
    followed by one free-axis reduce_min into G[:, t].
  - Epilogue: partition-fold RM via SBUF->SBUF DMA + tensor_tensor min
    (7 halvings), reduce-sum row/col results, one ones-matmul to sum
    across partitions, DMA the (1, 2) result out.
"""

import sys

import numpy as np

sys.path.insert(0, "/opt/trn_rl_repo")

import ml_dtypes  # noqa: E402

import concourse.bass as bass  # noqa: E402
import concourse.tile as tile  # noqa: E402
from concourse import bacc, mybir  # noqa: E402
from concourse import bass_utils  # noqa: E402

BF16 = ml_dtypes.bfloat16

B, N, M, D = 8, 8192, 8192, 3
K = 30          # augmented contract dim (10 rows per coordinate dim)
KP = 4          # PE row-group packing factor (4 concurrent matmuls)
KROWS = 32 * (KP - 1) + K   # 126: K rows replicated at 32-row strides
NT = N // 128   # 64 n-tiles
GW = 2048       # free-dim group width (4 psum banks)
NG = M // GW    # 4 groups per n-tile
MM = 512        # matmul free dim (1 psum bank)

_NC_CACHE = {}


def build_bass(n=N, m=M, gw=GW):
    f32 = mybir.dt.float32
    bf16 = mybir.dt.bfloat16
    MIN = mybir.AluOpType.min
    ADD = mybir.AluOpType.add
    AX = mybir.AxisListType.X

    nt = n // 128
    ng = m // gw

    nc = bacc.Bacc("TRN2", debug=False, num_devices=8)
    ahat_d = nc.dram_tensor("ahat", [KROWS, n], bf16, kind="ExternalInput")
    bhat_d = nc.dram_tensor("bhat", [KROWS, m], bf16, kind="ExternalInput")
    out_d = nc.dram_tensor("out", [1, 2], f32, kind="ExternalOutput")

    f16 = mybir.dt.float16  # min-path dtype: 10 mantissa bits, DVE 2x mode

    with tile.TileContext(nc) as tc:
        with (
            tc.tile_pool(name="inp", bufs=1) as inp_pool,
            tc.tile_pool(name="rm", bufs=1) as rm_pool,
            tc.tile_pool(name="x", bufs=2) as x_pool,
            tc.tile_pool(name="gt", bufs=1) as gt_pool,
            tc.tile_pool(name="fold", bufs=1) as fold_pool,
            tc.tile_pool(name="fin", bufs=1) as fin_pool,
            tc.tile_pool(name="carry", bufs=4) as carry_pool,
            tc.tile_pool(name="ps", bufs=2, space="PSUM") as ps_pool,
        ):
            ahat = inp_pool.tile([KROWS, n], bf16)
            bhat = inp_pool.tile([KROWS, m], bf16)
            for c in range(4):
                nc.sync.dma_start(ahat[:, bass.ts(c, n // 4)],
                                  ahat_d.ap()[:, bass.ts(c, n // 4)])
                nc.sync.dma_start(bhat[:, bass.ts(c, m // 4)],
                                  bhat_d.ap()[:, bass.ts(c, m // 4)])

            RM = rm_pool.tile([128, m], f16)        # running col-min
            G = gt_pool.tile([128, nt], f32)        # row-min per (p, t)

            for t in range(nt):
                X = x_pool.tile([128, m], f16, tag="x")
                for g in range(ng):
                    ps = ps_pool.tile([128, gw], f32, tag="ps")
                    for j in range(gw // MM):
                        mc = g * gw + j * MM
                        jp = 32 * (j % KP)
                        nc.tensor.matmul(
                            ps[:, bass.ts(j, MM)],
                            ahat[jp : jp + K, bass.ts(t, 128)],
                            bhat[jp : jp + K, mc : mc + MM],
                            start=True,
                            stop=True,
                            tile_position=(jp, 0),
                        )
                    # PSUM f32 -> SBUF f16
                    nc.scalar.copy(X[:, bass.ts(g, gw)], ps[:])
                # col-min accumulate: one big TT
                if t == 0:
                    nc.vector.tensor_copy(RM[:], X[:])
                else:
                    nc.vector.tensor_tensor(RM[:], RM[:], X[:], op=MIN)
                # row-min: in-place halving tree on X, then tiny reduce
                w = m // 2
                while w >= 128:
                    nc.vector.tensor_tensor(
                        X[:, 0:w], X[:, 0:w], X[:, w : 2 * w], op=MIN
                    )
                    w //= 2
                nc.vector.tensor_reduce(G[:, t : t + 1], X[:, 0:128], axis=AX, op=MIN)

            # ---- epilogue ----
            V = fin_pool.tile([128, 2], f32)
            ones = fin_pool.tile([128, 1], f32)
            nc.vector.memset(ones[:], 1.0)
            # sum of row-mins per partition
            nc.vector.tensor_reduce(V[:, 0:1], G[:], axis=AX, op=ADD)
            # fold col-min across partitions 128 -> 64, then route the
            # (64, m) remainder through DRAM so the per-m values land in
            # partition m%128; finish with a strided min-tree + sum.
            tmp = fold_pool.tile([64, m], f16, tag="fold")
            nc.sync.dma_start(tmp[:], RM[64:128, :])
            nc.vector.tensor_tensor(RM[0:64, :], RM[0:64, :], tmp[:], op=MIN)
            nq = 64
            with tc.tile_pool(name="dram", bufs=1, space="DRAM") as dram_pool:
                scr = dram_pool.tile([nq, m], f16)
                nc.sync.dma_start(scr[:], RM[0:nq, :])
                # colb[p, q, j] = scr[q, j*128 + p]
                colb = fold_pool.tile([128, nq, m // 128], f16, tag="colb")
                nc.sync.dma_start(
                    colb[:],
                    scr[:].rearrange("q (j p) -> p q j", p=128),
                )
                q = nq // 2
                while q >= 1:
                    nc.vector.tensor_tensor(
                        colb[:, 0:q, :], colb[:, 0:q, :], colb[:, q : 2 * q, :],
                        op=MIN,
                    )
                    q //= 2
                nc.vector.tensor_reduce(
                    V[:, 1:2], colb[:, 0, :], axis=AX, op=ADD
                )
            # cross-partition sum of V via ones-matmul
            outp = ps_pool.tile([1, 2], f32, tag="ps")
            nc.tensor.matmul(outp[:], ones[:], V[:], start=True, stop=True)
            osb = fin_pool.tile([1, 2], f32)
            nc.scalar.copy(osb[:], outp[:])
            nc.sync.dma_start(out_d.ap()[:, :], osb[:])

    nc.compile()
    return nc


def _get_nc():
    if "nc" not in _NC_CACHE:
        _NC_CACHE["nc"] = build_bass()
    return _NC_CACHE["nc"]


def _split2(x):
    """x -> (hi, lo) bf16 with hi+lo ~= x (~16 mantissa bits)."""
    hi = x.astype(BF16)
    lo = (x - hi.astype(x.dtype)).astype(BF16)
    return hi, lo


def _split3(x):
    """x (f64) -> (s1, s2, s3) bf16 with s1+s2+s3 ~= x (~24 bits)."""
    s1 = x.astype(BF16)
    r = x - s1.astype(x.dtype)
    s2 = r.astype(BF16)
    s3 = (r - s2.astype(x.dtype)).astype(BF16)
    return s1, s2, s3


def make_augmented(a, b):
    """a = gts[batch] (N,3) f32, b = preds[batch] (M,3) f32 ->
    ahat (30,N) bf16, bhat (30,M) bf16 with ahat.T @ bhat ~= d2.

    Per coordinate dim d (10 rows): with a_r = ahi+alo, q_r = qhi+qlo
    (q = -2b), na_d = a_r^2 (3-split), nb_d = (q_r/2)^2 (3-split):
      na_d + nb_d + a_r*q_r = (a_r - q_r/(-2))^2... i.e. the exact
      per-dim squared difference of the bf16-represented points.
    Partial sums stay O(coord^2), keeping f32 PSUM rounding tiny.
    """
    a = np.asarray(a, np.float32)
    b = np.asarray(b, np.float32)
    q = (-2.0 * b).astype(np.float32)
    ahi, alo = _split2(a)
    qhi, qlo = _split2(q)
    a_r = ahi.astype(np.float64) + alo.astype(np.float64)   # (N,3)
    q_r = qhi.astype(np.float64) + qlo.astype(np.float64)   # (M,3)
    one_a = np.ones(a.shape[0], BF16)
    one_b = np.ones(b.shape[0], BF16)
    arows = []
    brows = []
    for d in range(3):
        na1, na2, na3 = _split3(a_r[:, d] ** 2)
        nb1, nb2, nb3 = _split3((q_r[:, d] * 0.5) ** 2)
        arows += [na1, na2, na3, one_a, one_a, one_a,
                  ahi[:, d], alo[:, d], ahi[:, d], alo[:, d]]
        brows += [one_b, one_b, one_b, nb1, nb2, nb3,
                  qhi[:, d], qhi[:, d], qlo[:, d], qlo[:, d]]
    ahat = np.stack(arows)
    bhat = np.stack(brows)
    return _replicate_rows(ahat), _replicate_rows(bhat)


def _replicate_rows(x):
    """(K, n) -> (KROWS, n): copies at 32-row strides for PE row-group
    packing (4 concurrent matmuls in one array pass)."""
    out = np.zeros((KROWS, x.shape[1]), BF16)
    for j in range(KP):
        out[32 * j : 32 * j + K] = x
    return np.ascontiguousarray(out)


def make_in_maps(gts, preds):
    in_maps = []
    for b in range(B):
        ahat, bhat = make_augmented(gts[b], preds[b])
        in_maps.append({"ahat": ahat, "bhat": bhat})
    return in_maps


def run_spmd(gts, preds, trace=False):
    nc = _get_nc()
    in_maps = make_in_maps(gts, preds)
    res = bass_utils.run_bass_kernel_spmd(
        nc, in_maps, core_ids=list(range(B)), trace=trace
    )
    return res


def _combine(results):
    tot = 0.0
    for r in results:
        o = np.asarray(r["out"], np.float64)
        tot += o[0, 0] + o[0, 1]
    return np.float32(tot / (B * N))


def kernel(gts, preds):
    res = run_spmd(np.asarray(gts), np.asarray(preds), trace=False)
    return np.asarray(_combine(res.results))


# revision 22
# speedup vs baseline: 1.8450x; 1.8450x over previous
"""Chamfer loss kernel for 8x TRN2 NeuronCores.

Problem: gts (8, 8192, 3) f32, preds (8, 8192, 3) f32 ->
    scalar = mean_n min_m d2[b,n,m] + mean_m min_n d2[b,n,m]
where d2 = squared euclidean distance.

Sharding: data-parallel over batch B=8, one batch element per core. Each
core evaluates its full 8192x8192 distance block and reduces it to two
partial sums (sum of row-mins, sum of col-mins); the host sums the 8x2
partials and divides by B*N.

Device algorithm (per core):
  - d2 is produced by ONE bf16 matmul per (128n x 512m) tile using an
    augmented split-bf16 embedding with contract dim K=16:
        ahat = [a_hi(3), a_lo(3), a_hi(3), a_lo(3), na_hi, na_lo, 1, 1]
        bhat = [q_hi(3), q_hi(3), q_lo(3), q_lo(3), 1, 1, nb_hi, nb_lo]
    with q = -2*preds, na = |a|^2, nb = |b|^2, each value split into
    hi/lo bf16 parts.  ahat.T @ bhat = |a|^2 + |b|^2 - 2 a.b to ~2^-16
    relative accuracy (PSUM accumulates in f32), i.e. f32-grade d2 at
    bf16 streaming speed.
  - ScalarE copies each PSUM group (128 x 2048 f32) to SBUF as bf16.
  - VectorE does both min reductions in bf16 (tensor_tensor min = 2x
    mode): a running col-min accumulator RM (128 x 8192) folded over
    n-tiles, and a per-n-tile row accumulator R folded over m-groups
    followed by one free-axis reduce_min into G[:, t].
  - Epilogue: partition-fold RM via SBUF->SBUF DMA + tensor_tensor min
    (7 halvings), reduce-sum row/col results, one ones-matmul to sum
    across partitions, DMA the (1, 2) result out.
"""

import sys

import numpy as np

sys.path.insert(0, "/opt/trn_rl_repo")

import ml_dtypes  # noqa: E402

import concourse.bass as bass  # noqa: E402
import concourse.tile as tile  # noqa: E402
from concourse import bacc, mybir  # noqa: E402
from concourse import bass_utils  # noqa: E402

BF16 = ml_dtypes.bfloat16

B, N, M, D = 8, 8192, 8192, 3
K = 30          # augmented contract dim (10 rows per coordinate dim)
KP = 4          # PE row-group packing factor (4 concurrent matmuls)
KROWS = 32 * (KP - 1) + K   # 126: K rows replicated at 32-row strides
NT = N // 128   # 64 n-tiles
GW = 2048       # free-dim group width (4 psum banks)
NG = M // GW    # 4 groups per n-tile
MM = 512        # matmul free dim (1 psum bank)

_NC_CACHE = {}


def build_bass(n=N, m=M, gw=GW):
    f32 = mybir.dt.float32
    bf16 = mybir.dt.bfloat16
    MIN = mybir.AluOpType.min
    ADD = mybir.AluOpType.add
    AX = mybir.AxisListType.X

    nt = n // 128
    ng = m // gw

    nc = bacc.Bacc("TRN2", debug=False, num_devices=8)
    ahat_d = nc.dram_tensor("ahat", [KROWS, n], bf16, kind="ExternalInput")
    bhat_d = nc.dram_tensor("bhat", [KROWS, m], bf16, kind="ExternalInput")
    out_d = nc.dram_tensor("out", [1, 2], f32, kind="ExternalOutput")

    f16 = mybir.dt.float16  # min-path dtype: 10 mantissa bits, DVE 2x mode

    with tile.TileContext(nc) as tc:
        with (
            tc.tile_pool(name="inp", bufs=1) as inp_pool,
            tc.tile_pool(name="rm", bufs=1) as rm_pool,
            tc.tile_pool(name="x", bufs=3) as x_pool,
            tc.tile_pool(name="gt", bufs=1) as gt_pool,
            tc.tile_pool(name="fold", bufs=1) as fold_pool,
            tc.tile_pool(name="fin", bufs=1) as fin_pool,
            tc.tile_pool(name="carry", bufs=4) as carry_pool,
            tc.tile_pool(name="ps", bufs=2, space="PSUM") as ps_pool,
        ):
            ahat = inp_pool.tile([KROWS, n], bf16)
            bhat = inp_pool.tile([KROWS, m], bf16)
            for c in range(4):
                nc.sync.dma_start(ahat[:, bass.ts(c, n // 4)],
                                  ahat_d.ap()[:, bass.ts(c, n // 4)])
                nc.sync.dma_start(bhat[:, bass.ts(c, m // 4)],
                                  bhat_d.ap()[:, bass.ts(c, m // 4)])

            RM = rm_pool.tile([128, m], f16)        # running col-min
            G = gt_pool.tile([128, nt], f32)        # row-min per (p, t)

            for t in range(nt):
                X = x_pool.tile([128, m], f16, tag="x")
                for g in range(ng):
                    ps = ps_pool.tile([128, gw], f32, tag="ps")
                    for j in range(gw // MM):
                        mc = g * gw + j * MM
                        jp = 32 * (j % KP)
                        nc.tensor.matmul(
                            ps[:, bass.ts(j, MM)],
                            ahat[jp : jp + K, bass.ts(t, 128)],
                            bhat[jp : jp + K, mc : mc + MM],
                            start=True,
                            stop=True,
                            tile_position=(jp, 0),
                        )
                    # PSUM f32 -> SBUF f16
                    nc.scalar.copy(X[:, bass.ts(g, gw)], ps[:])
                # col-min accumulate: one big TT
                if t == 0:
                    nc.vector.tensor_copy(RM[:], X[:])
                else:
                    nc.vector.tensor_tensor(RM[:], RM[:], X[:], op=MIN)
                # row-min: in-place halving tree on X, then tiny reduce
                w = m // 2
                while w >= 128:
                    nc.vector.tensor_tensor(
                        X[:, 0:w], X[:, 0:w], X[:, w : 2 * w], op=MIN
                    )
                    w //= 2
                nc.vector.tensor_reduce(G[:, t : t + 1], X[:, 0:128], axis=AX, op=MIN)

            # ---- epilogue ----
            V = fin_pool.tile([128, 2], f32)
            ones = fin_pool.tile([128, 1], f32)
            nc.vector.memset(ones[:], 1.0)
            # sum of row-mins per partition
            nc.vector.tensor_reduce(V[:, 0:1], G[:], axis=AX, op=ADD)
            # col-min across partitions: xbar-transpose RM in 128x128
            # blocks (RT[p, c, q] = RM[q, c*128+p]), then min-tree over q
            # and a final sum over the m's owned by each partition.
            nblk = m // 128
            RT = fold_pool.tile([128, nblk, 128], f16, tag="fold")
            for c in range(nblk):
                nc.sync.dma_start_transpose(
                    RT[:, c, :], RM[:, bass.ts(c, 128)]
                )
            q = 64
            while q >= 1:
                nc.vector.tensor_tensor(
                    RT[:, :, 0:q], RT[:, :, 0:q], RT[:, :, q : 2 * q], op=MIN
                )
                q //= 2
            nc.vector.tensor_reduce(V[:, 1:2], RT[:, :, 0], axis=AX, op=ADD)
            # cross-partition sum of V via ones-matmul
            outp = ps_pool.tile([1, 2], f32, tag="ps")
            nc.tensor.matmul(outp[:], ones[:], V[:], start=True, stop=True)
            osb = fin_pool.tile([1, 2], f32)
            nc.scalar.copy(osb[:], outp[:])
            nc.sync.dma_start(out_d.ap()[:, :], osb[:])

    nc.compile()
    return nc


def _get_nc():
    if "nc" not in _NC_CACHE:
        _NC_CACHE["nc"] = build_bass()
    return _NC_CACHE["nc"]


def _split2(x):
    """x -> (hi, lo) bf16 with hi+lo ~= x (~16 mantissa bits)."""
    hi = x.astype(BF16)
    lo = (x - hi.astype(x.dtype)).astype(BF16)
    return hi, lo


def _split3(x):
    """x (f64) -> (s1, s2, s3) bf16 with s1+s2+s3 ~= x (~24 bits)."""
    s1 = x.astype(BF16)
    r = x - s1.astype(x.dtype)
    s2 = r.astype(BF16)
    s3 = (r - s2.astype(x.dtype)).astype(BF16)
    return s1, s2, s3


def make_augmented(a, b):
    """a = gts[batch] (N,3) f32, b = preds[batch] (M,3) f32 ->
    ahat (30,N) bf16, bhat (30,M) bf16 with ahat.T @ bhat ~= d2.

    Per coordinate dim d (10 rows): with a_r = ahi+alo, q_r = qhi+qlo
    (q = -2b), na_d = a_r^2 (3-split), nb_d = (q_r/2)^2 (3-split):
      na_d + nb_d + a_r*q_r = (a_r - q_r/(-2))^2... i.e. the exact
      per-dim squared difference of the bf16-represented points.
    Partial sums stay O(coord^2), keeping f32 PSUM rounding tiny.
    """
    a = np.asarray(a, np.float32)
    b = np.asarray(b, np.float32)
    q = (-2.0 * b).astype(np.float32)
    ahi, alo = _split2(a)
    qhi, qlo = _split2(q)
    a_r = ahi.astype(np.float64) + alo.astype(np.float64)   # (N,3)
    q_r = qhi.astype(np.float64) + qlo.astype(np.float64)   # (M,3)
    one_a = np.ones(a.shape[0], BF16)
    one_b = np.ones(b.shape[0], BF16)
    arows = []
    brows = []
    for d in range(3):
        na1, na2, na3 = _split3(a_r[:, d] ** 2)
        nb1, nb2, nb3 = _split3((q_r[:, d] * 0.5) ** 2)
        arows += [na1, na2, na3, one_a, one_a, one_a,
                  ahi[:, d], alo[:, d], ahi[:, d], alo[:, d]]
        brows += [one_b, one_b, one_b, nb1, nb2, nb3,
                  qhi[:, d], qhi[:, d], qlo[:, d], qlo[:, d]]
    ahat = np.stack(arows)
    bhat = np.stack(brows)
    return _replicate_rows(ahat), _replicate_rows(bhat)


def _replicate_rows(x):
    """(K, n) -> (KROWS, n): copies at 32-row strides for PE row-group
    packing (4 concurrent matmuls in one array pass)."""
    out = np.zeros((KROWS, x.shape[1]), BF16)
    for j in range(KP):
        out[32 * j : 32 * j + K] = x
    return np.ascontiguousarray(out)


def make_in_maps(gts, preds):
    in_maps = []
    for b in range(B):
        ahat, bhat = make_augmented(gts[b], preds[b])
        in_maps.append({"ahat": ahat, "bhat": bhat})
    return in_maps


def run_spmd(gts, preds, trace=False):
    nc = _get_nc()
    in_maps = make_in_maps(gts, preds)
    res = bass_utils.run_bass_kernel_spmd(
        nc, in_maps, core_ids=list(range(B)), trace=trace
    )
    return res


def _combine(results):
    tot = 0.0
    for r in results:
        o = np.asarray(r["out"], np.float64)
        tot += o[0, 0] + o[0, 1]
    return np.float32(tot / (B * N))


def kernel(gts, preds):
    res = run_spmd(np.asarray(gts), np.asarray(preds), trace=False)
    return np.asarray(_combine(res.results))


# revision 24
# speedup vs baseline: 2.0556x; 1.1141x over previous
"""Chamfer loss kernel for 8x TRN2 NeuronCores.

Problem: gts (8, 8192, 3) f32, preds (8, 8192, 3) f32 ->
    scalar = mean_n min_m d2[b,n,m] + mean_m min_n d2[b,n,m]
where d2 = squared euclidean distance.

Sharding: data-parallel over batch B=8, one batch element per core. Each
core evaluates its full 8192x8192 distance block and reduces it to two
partial sums (sum of row-mins, sum of col-mins); the host sums the 8x2
partials and divides by B*N.

Device algorithm (per core):
  - d2 is produced by ONE bf16 matmul per (128n x 512m) tile using an
    augmented split-bf16 embedding with contract dim K=16:
        ahat = [a_hi(3), a_lo(3), a_hi(3), a_lo(3), na_hi, na_lo, 1, 1]
        bhat = [q_hi(3), q_hi(3), q_lo(3), q_lo(3), 1, 1, nb_hi, nb_lo]
    with q = -2*preds, na = |a|^2, nb = |b|^2, each value split into
    hi/lo bf16 parts.  ahat.T @ bhat = |a|^2 + |b|^2 - 2 a.b to ~2^-16
    relative accuracy (PSUM accumulates in f32), i.e. f32-grade d2 at
    bf16 streaming speed.
  - ScalarE copies each PSUM group (128 x 2048 f32) to SBUF as bf16.
  - VectorE does both min reductions in bf16 (tensor_tensor min = 2x
    mode): a running col-min accumulator RM (128 x 8192) folded over
    n-tiles, and a per-n-tile row accumulator R folded over m-groups
    followed by one free-axis reduce_min into G[:, t].
  - Epilogue: partition-fold RM via SBUF->SBUF DMA + tensor_tensor min
    (7 halvings), reduce-sum row/col results, one ones-matmul to sum
    across partitions, DMA the (1, 2) result out.
"""

import sys

import numpy as np

sys.path.insert(0, "/opt/trn_rl_repo")

import ml_dtypes  # noqa: E402

import concourse.bass as bass  # noqa: E402
import concourse.tile as tile  # noqa: E402
from concourse import bacc, mybir  # noqa: E402
from concourse import bass_utils  # noqa: E402

BF16 = ml_dtypes.bfloat16

B, N, M, D = 8, 8192, 8192, 3
K = 30          # augmented contract dim (10 rows per coordinate dim)
KP = 4          # PE row-group packing factor (4 concurrent matmuls)
KROWS = 32 * (KP - 1) + K   # 126: K rows replicated at 32-row strides
NT = N // 128   # 64 n-tiles
GW = 2048       # free-dim group width (4 psum banks)
NG = M // GW    # 4 groups per n-tile
MM = 512        # matmul free dim (1 psum bank)

_NC_CACHE = {}


def build_bass(n=N, m=M, gw=GW):
    f32 = mybir.dt.float32
    bf16 = mybir.dt.bfloat16
    MIN = mybir.AluOpType.min
    ADD = mybir.AluOpType.add
    AX = mybir.AxisListType.X

    nt = n // 128
    ng = m // gw

    nc = bacc.Bacc("TRN2", debug=False, num_devices=8)
    ahat_d = nc.dram_tensor("ahat", [KROWS, n], bf16, kind="ExternalInput")
    bhat_d = nc.dram_tensor("bhat", [KROWS, m], bf16, kind="ExternalInput")
    out_d = nc.dram_tensor("out", [1, 2], f32, kind="ExternalOutput")

    f16 = mybir.dt.float16  # min-path dtype: 10 mantissa bits, DVE 2x mode

    with tile.TileContext(nc) as tc:
        with (
            tc.tile_pool(name="inp", bufs=1) as inp_pool,
            tc.tile_pool(name="rm", bufs=1) as rm_pool,
            tc.tile_pool(name="x", bufs=3) as x_pool,
            tc.tile_pool(name="gt", bufs=1) as gt_pool,
            tc.tile_pool(name="fold", bufs=1) as fold_pool,
            tc.tile_pool(name="fin", bufs=1) as fin_pool,
            tc.tile_pool(name="carry", bufs=4) as carry_pool,
            tc.tile_pool(name="ps", bufs=2, space="PSUM") as ps_pool,
        ):
            ahat = inp_pool.tile([KROWS, n], bf16)
            bhat = inp_pool.tile([KROWS, m], bf16)
            for c in range(4):
                nc.scalar.dma_start(ahat[:, bass.ts(c, n // 4)],
                                    ahat_d.ap()[:, bass.ts(c, n // 4)])
                nc.sync.dma_start(bhat[:, bass.ts(c, m // 4)],
                                  bhat_d.ap()[:, bass.ts(c, m // 4)])

            RM = rm_pool.tile([128, m], f16)        # running col-min
            G = gt_pool.tile([128, nt], f32)        # row-min per (p, t)

            for t in range(nt):
                X = x_pool.tile([128, m], f16, tag="x")
                for g in range(ng):
                    ps = ps_pool.tile([128, gw], f32, tag="ps")
                    for j in range(gw // MM):
                        mc = g * gw + j * MM
                        jp = 32 * (j % KP)
                        nc.tensor.matmul(
                            ps[:, bass.ts(j, MM)],
                            ahat[jp : jp + K, bass.ts(t, 128)],
                            bhat[jp : jp + K, mc : mc + MM],
                            start=True,
                            stop=True,
                            tile_position=(jp, 0),
                        )
                    # PSUM f32 -> SBUF f16
                    nc.scalar.copy(X[:, bass.ts(g, gw)], ps[:])
                # col-min accumulate: one big TT
                if t == 0:
                    nc.vector.tensor_copy(RM[:], X[:])
                else:
                    nc.vector.tensor_tensor(RM[:], RM[:], X[:], op=MIN)
                # row-min: in-place halving tree on X, then tiny reduce
                w = m // 2
                while w >= 128:
                    nc.vector.tensor_tensor(
                        X[:, 0:w], X[:, 0:w], X[:, w : 2 * w], op=MIN
                    )
                    w //= 2
                nc.vector.tensor_reduce(G[:, t : t + 1], X[:, 0:128], axis=AX, op=MIN)

            # ---- epilogue ----
            V = fin_pool.tile([128, 2], f32)
            ones = fin_pool.tile([128, 1], f32)
            nc.vector.memset(ones[:], 1.0)
            # sum of row-mins per partition
            nc.vector.tensor_reduce(V[:, 0:1], G[:], axis=AX, op=ADD)
            # col-min across partitions: xbar-transpose RM in 128x128
            # blocks (RT[p, c, q] = RM[q, c*128+p]), then min-tree over q
            # and a final sum over the m's owned by each partition.
            nblk = m // 128
            RT = fold_pool.tile([128, nblk, 128], f16, tag="fold")
            nc.sync.dma_start_transpose(RT[:], RM[:])
            q = 64
            while q >= 1:
                nc.vector.tensor_tensor(
                    RT[:, :, 0:q], RT[:, :, 0:q], RT[:, :, q : 2 * q], op=MIN
                )
                q //= 2
            nc.vector.tensor_reduce(V[:, 1:2], RT[:, :, 0], axis=AX, op=ADD)
            # cross-partition sum of V via ones-matmul
            outp = ps_pool.tile([1, 2], f32, tag="ps")
            nc.tensor.matmul(outp[:], ones[:], V[:], start=True, stop=True)
            osb = fin_pool.tile([1, 2], f32)
            nc.scalar.copy(osb[:], outp[:])
            nc.sync.dma_start(out_d.ap()[:, :], osb[:])

    nc.compile()
    return nc


def _get_nc():
    if "nc" not in _NC_CACHE:
        _NC_CACHE["nc"] = build_bass()
    return _NC_CACHE["nc"]


def _split2(x):
    """x -> (hi, lo) bf16 with hi+lo ~= x (~16 mantissa bits)."""
    hi = x.astype(BF16)
    lo = (x - hi.astype(x.dtype)).astype(BF16)
    return hi, lo


def _split3(x):
    """x (f64) -> (s1, s2, s3) bf16 with s1+s2+s3 ~= x (~24 bits)."""
    s1 = x.astype(BF16)
    r = x - s1.astype(x.dtype)
    s2 = r.astype(BF16)
    s3 = (r - s2.astype(x.dtype)).astype(BF16)
    return s1, s2, s3


def make_augmented(a, b):
    """a = gts[batch] (N,3) f32, b = preds[batch] (M,3) f32 ->
    ahat (30,N) bf16, bhat (30,M) bf16 with ahat.T @ bhat ~= d2.

    Per coordinate dim d (10 rows): with a_r = ahi+alo, q_r = qhi+qlo
    (q = -2b), na_d = a_r^2 (3-split), nb_d = (q_r/2)^2 (3-split):
      na_d + nb_d + a_r*q_r = (a_r - q_r/(-2))^2... i.e. the exact
      per-dim squared difference of the bf16-represented points.
    Partial sums stay O(coord^2), keeping f32 PSUM rounding tiny.
    """
    a = np.asarray(a, np.float32)
    b = np.asarray(b, np.float32)
    q = (-2.0 * b).astype(np.float32)
    ahi, alo = _split2(a)
    qhi, qlo = _split2(q)
    a_r = ahi.astype(np.float64) + alo.astype(np.float64)   # (N,3)
    q_r = qhi.astype(np.float64) + qlo.astype(np.float64)   # (M,3)
    one_a = np.ones(a.shape[0], BF16)
    one_b = np.ones(b.shape[0], BF16)
    arows = []
    brows = []
    for d in range(3):
        na1, na2, na3 = _split3(a_r[:, d] ** 2)
        nb1, nb2, nb3 = _split3((q_r[:, d] * 0.5) ** 2)
        arows += [na1, na2, na3, one_a, one_a, one_a,
                  ahi[:, d], alo[:, d], ahi[:, d], alo[:, d]]
        brows += [one_b, one_b, one_b, nb1, nb2, nb3,
                  qhi[:, d], qhi[:, d], qlo[:, d], qlo[:, d]]
    ahat = np.stack(arows)
    bhat = np.stack(brows)
    return _replicate_rows(ahat), _replicate_rows(bhat)


def _replicate_rows(x):
    """(K, n) -> (KROWS, n): copies at 32-row strides for PE row-group
    packing (4 concurrent matmuls in one array pass)."""
    out = np.zeros((KROWS, x.shape[1]), BF16)
    for j in range(KP):
        out[32 * j : 32 * j + K] = x
    return np.ascontiguousarray(out)


def make_in_maps(gts, preds):
    in_maps = []
    for b in range(B):
        ahat, bhat = make_augmented(gts[b], preds[b])
        in_maps.append({"ahat": ahat, "bhat": bhat})
    return in_maps


def run_spmd(gts, preds, trace=False):
    nc = _get_nc()
    in_maps = make_in_maps(gts, preds)
    res = bass_utils.run_bass_kernel_spmd(
        nc, in_maps, core_ids=list(range(B)), trace=trace
    )
    return res


def _combine(results):
    tot = 0.0
    for r in results:
        o = np.asarray(r["out"], np.float64)
        tot += o[0, 0] + o[0, 1]
    return np.float32(tot / (B * N))


def kernel(gts, preds):
    res = run_spmd(np.asarray(gts), np.asarray(preds), trace=False)
    return np.asarray(_combine(res.results))


# revision 27
# speedup vs baseline: 2.0700x; 1.0070x over previous
"""Chamfer loss kernel for 8x TRN2 NeuronCores.

Problem: gts (8, 8192, 3) f32, preds (8, 8192, 3) f32 ->
    scalar = mean_n min_m d2[b,n,m] + mean_m min_n d2[b,n,m]
where d2 = squared euclidean distance.

Sharding: data-parallel over batch B=8, one batch element per core. Each
core evaluates its full 8192x8192 distance block and reduces it to two
partial sums (sum of row-mins, sum of col-mins); the host sums the 8x2
partials and divides by B*N.

Device algorithm (per core):
  - d2 is produced by ONE bf16 matmul per (128n x 512m) tile using an
    augmented split-bf16 embedding with contract dim K=16:
        ahat = [a_hi(3), a_lo(3), a_hi(3), a_lo(3), na_hi, na_lo, 1, 1]
        bhat = [q_hi(3), q_hi(3), q_lo(3), q_lo(3), 1, 1, nb_hi, nb_lo]
    with q = -2*preds, na = |a|^2, nb = |b|^2, each value split into
    hi/lo bf16 parts.  ahat.T @ bhat = |a|^2 + |b|^2 - 2 a.b to ~2^-16
    relative accuracy (PSUM accumulates in f32), i.e. f32-grade d2 at
    bf16 streaming speed.
  - ScalarE copies each PSUM group (128 x 2048 f32) to SBUF as bf16.
  - VectorE does both min reductions in bf16 (tensor_tensor min = 2x
    mode): a running col-min accumulator RM (128 x 8192) folded over
    n-tiles, and a per-n-tile row accumulator R folded over m-groups
    followed by one free-axis reduce_min into G[:, t].
  - Epilogue: partition-fold RM via SBUF->SBUF DMA + tensor_tensor min
    (7 halvings), reduce-sum row/col results, one ones-matmul to sum
    across partitions, DMA the (1, 2) result out.
"""

import sys

import numpy as np

sys.path.insert(0, "/opt/trn_rl_repo")

import ml_dtypes  # noqa: E402

import concourse.bass as bass  # noqa: E402
import concourse.tile as tile  # noqa: E402
from concourse import bacc, mybir  # noqa: E402
from concourse import bass_utils  # noqa: E402

BF16 = ml_dtypes.bfloat16

B, N, M, D = 8, 8192, 8192, 3
K = 30          # augmented contract dim (10 rows per coordinate dim)
KP = 4          # PE row-group packing factor (4 concurrent matmuls)
KROWS = 32 * (KP - 1) + K   # 126: K rows replicated at 32-row strides
NT = N // 128   # 64 n-tiles
GW = 2048       # free-dim group width (4 psum banks)
NG = M // GW    # 4 groups per n-tile
MM = 512        # matmul free dim (1 psum bank)

_NC_CACHE = {}


def build_bass(n=N, m=M, gw=GW):
    f32 = mybir.dt.float32
    bf16 = mybir.dt.bfloat16
    MIN = mybir.AluOpType.min
    ADD = mybir.AluOpType.add
    AX = mybir.AxisListType.X

    nt = n // 128
    ng = m // gw

    nc = bacc.Bacc("TRN2", debug=False, num_devices=8)
    ahat_d = nc.dram_tensor("ahat", [KROWS, n], bf16, kind="ExternalInput")
    bhat_d = nc.dram_tensor("bhat", [KROWS, m], bf16, kind="ExternalInput")
    out_d = nc.dram_tensor("out", [1, 2], f32, kind="ExternalOutput")

    f16 = mybir.dt.float16  # min-path dtype: 10 mantissa bits, DVE 2x mode

    with tile.TileContext(nc) as tc:
        with (
            tc.tile_pool(name="inp", bufs=1) as inp_pool,
            tc.tile_pool(name="rm", bufs=1) as rm_pool,
            tc.tile_pool(name="x", bufs=3) as x_pool,
            tc.tile_pool(name="gt", bufs=1) as gt_pool,
            tc.tile_pool(name="fold", bufs=1) as fold_pool,
            tc.tile_pool(name="fin", bufs=1) as fin_pool,
            tc.tile_pool(name="carry", bufs=4) as carry_pool,
            tc.tile_pool(name="ps", bufs=2, space="PSUM") as ps_pool,
        ):
            ahat = inp_pool.tile([KROWS, n], bf16)
            bhat = inp_pool.tile([KROWS, m], bf16)
            # bhat gates the first tile's compute: split it finely across
            # both HWDGE queues; ahat chunk 0 first so matmuls can start.
            nc.scalar.dma_start(ahat[:, 0 : n // 4],
                                ahat_d.ap()[:, 0 : n // 4])
            for c in range(8):
                eng = nc.sync if c % 2 == 0 else nc.scalar
                eng.dma_start(bhat[:, bass.ts(c, m // 8)],
                              bhat_d.ap()[:, bass.ts(c, m // 8)])
            for c in range(1, 4):
                nc.sync.dma_start(ahat[:, bass.ts(c, n // 4)],
                                  ahat_d.ap()[:, bass.ts(c, n // 4)])

            RM = rm_pool.tile([128, m], f16)        # running col-min
            G = gt_pool.tile([128, nt], f32)        # row-min per (p, t)

            for t in range(nt):
                X = x_pool.tile([128, m], f16, tag="x")
                for g in range(ng):
                    ps = ps_pool.tile([128, gw], f32, tag="ps")
                    for j in range(gw // MM):
                        mc = g * gw + j * MM
                        jp = 32 * (j % KP)
                        nc.tensor.matmul(
                            ps[:, bass.ts(j, MM)],
                            ahat[jp : jp + K, bass.ts(t, 128)],
                            bhat[jp : jp + K, mc : mc + MM],
                            start=True,
                            stop=True,
                            tile_position=(jp, 0),
                        )
                    # PSUM f32 -> SBUF f16
                    nc.scalar.copy(X[:, bass.ts(g, gw)], ps[:])
                # col-min accumulate: one big TT
                if t == 0:
                    nc.vector.tensor_copy(RM[:], X[:])
                else:
                    nc.vector.tensor_tensor(RM[:], RM[:], X[:], op=MIN)
                # row-min: in-place halving tree on X, then tiny reduce
                w = m // 2
                while w >= 128:
                    nc.vector.tensor_tensor(
                        X[:, 0:w], X[:, 0:w], X[:, w : 2 * w], op=MIN
                    )
                    w //= 2
                nc.vector.tensor_reduce(G[:, t : t + 1], X[:, 0:128], axis=AX, op=MIN)

            # ---- epilogue ----
            V = fin_pool.tile([128, 2], f32)
            ones = fin_pool.tile([128, 1], f32)
            nc.vector.memset(ones[:], 1.0)
            # sum of row-mins per partition
            nc.vector.tensor_reduce(V[:, 0:1], G[:], axis=AX, op=ADD)
            # col-min across partitions: xbar-transpose RM in 128x128
            # blocks (RT[p, c, q] = RM[q, c*128+p]), then min-tree over q
            # and a final sum over the m's owned by each partition.
            nblk = m // 128
            RT = fold_pool.tile([128, nblk, 128], f16, tag="fold")
            nc.sync.dma_start_transpose(RT[:], RM[:])
            q = 64
            while q >= 1:
                nc.vector.tensor_tensor(
                    RT[:, :, 0:q], RT[:, :, 0:q], RT[:, :, q : 2 * q], op=MIN
                )
                q //= 2
            nc.vector.tensor_reduce(V[:, 1:2], RT[:, :, 0], axis=AX, op=ADD)
            # cross-partition sum of V via ones-matmul
            outp = ps_pool.tile([1, 2], f32, tag="ps")
            nc.tensor.matmul(outp[:], ones[:], V[:], start=True, stop=True)
            osb = fin_pool.tile([1, 2], f32)
            nc.scalar.copy(osb[:], outp[:])
            nc.sync.dma_start(out_d.ap()[:, :], osb[:])

    nc.compile()
    return nc


def _get_nc():
    if "nc" not in _NC_CACHE:
        _NC_CACHE["nc"] = build_bass()
    return _NC_CACHE["nc"]


def _split2(x):
    """x -> (hi, lo) bf16 with hi+lo ~= x (~16 mantissa bits)."""
    hi = x.astype(BF16)
    lo = (x - hi.astype(x.dtype)).astype(BF16)
    return hi, lo


def _split3(x):
    """x (f64) -> (s1, s2, s3) bf16 with s1+s2+s3 ~= x (~24 bits)."""
    s1 = x.astype(BF16)
    r = x - s1.astype(x.dtype)
    s2 = r.astype(BF16)
    s3 = (r - s2.astype(x.dtype)).astype(BF16)
    return s1, s2, s3


def make_augmented(a, b):
    """a = gts[batch] (N,3) f32, b = preds[batch] (M,3) f32 ->
    ahat (30,N) bf16, bhat (30,M) bf16 with ahat.T @ bhat ~= d2.

    Per coordinate dim d (10 rows): with a_r = ahi+alo, q_r = qhi+qlo
    (q = -2b), na_d = a_r^2 (3-split), nb_d = (q_r/2)^2 (3-split):
      na_d + nb_d + a_r*q_r = (a_r - q_r/(-2))^2... i.e. the exact
      per-dim squared difference of the bf16-represented points.
    Partial sums stay O(coord^2), keeping f32 PSUM rounding tiny.
    """
    a = np.asarray(a, np.float32)
    b = np.asarray(b, np.float32)
    q = (-2.0 * b).astype(np.float32)
    ahi, alo = _split2(a)
    qhi, qlo = _split2(q)
    a_r = ahi.astype(np.float64) + alo.astype(np.float64)   # (N,3)
    q_r = qhi.astype(np.float64) + qlo.astype(np.float64)   # (M,3)
    one_a = np.ones(a.shape[0], BF16)
    one_b = np.ones(b.shape[0], BF16)
    arows = []
    brows = []
    for d in range(3):
        na1, na2, na3 = _split3(a_r[:, d] ** 2)
        nb1, nb2, nb3 = _split3((q_r[:, d] * 0.5) ** 2)
        arows += [na1, na2, na3, one_a, one_a, one_a,
                  ahi[:, d], alo[:, d], ahi[:, d], alo[:, d]]
        brows += [one_b, one_b, one_b, nb1, nb2, nb3,
                  qhi[:, d], qhi[:, d], qlo[:, d], qlo[:, d]]
    ahat = np.stack(arows)
    bhat = np.stack(brows)
    return _replicate_rows(ahat), _replicate_rows(bhat)


def _replicate_rows(x):
    """(K, n) -> (KROWS, n): copies at 32-row strides for PE row-group
    packing (4 concurrent matmuls in one array pass)."""
    out = np.zeros((KROWS, x.shape[1]), BF16)
    for j in range(KP):
        out[32 * j : 32 * j + K] = x
    return np.ascontiguousarray(out)


def make_in_maps(gts, preds):
    in_maps = []
    for b in range(B):
        ahat, bhat = make_augmented(gts[b], preds[b])
        in_maps.append({"ahat": ahat, "bhat": bhat})
    return in_maps


def run_spmd(gts, preds, trace=False):
    nc = _get_nc()
    in_maps = make_in_maps(gts, preds)
    res = bass_utils.run_bass_kernel_spmd(
        nc, in_maps, core_ids=list(range(B)), trace=trace
    )
    return res


def _combine(results):
    tot = 0.0
    for r in results:
        o = np.asarray(r["out"], np.float64)
        tot += o[0, 0] + o[0, 1]
    return np.float32(tot / (B * N))


def kernel(gts, preds):
    res = run_spmd(np.asarray(gts), np.asarray(preds), trace=False)
    return np.asarray(_combine(res.results))
